# revision 19
# baseline (speedup 1.0000x reference)
"""Trainium2 Bass kernel for batched 64-point DCT (flattened-patch GEMM).

Reference computation: out = x.reshape(b, -1, 64) @ K, reshaped back.
Pure data parallel over 8 NeuronCores: core i handles batch i as a
[49152, 64] x [64, 64] GEMM.

Sharding strategy: while slicing the batch on the host, each core's input
is also laid out as xth[(z, s), pair] = x[2*pair + z, s] -- a [128, 24576]
matrix whose partition dim is (patch-parity, DCT-coefficient). With that
layout the device kernel needs no on-chip transposes at all:

  1. DMA in [128, 2048] tiles (8 KB contiguous per partition).
  2. One fp32 matmul per 128 pair-columns: stationary = data slice
     [128, 128], moving = blockdiag(K, K) [128, 128]:
       out[q, z*64+f] = sum_s x[2*(base+q)+z, s] * K[s, f]
     which is exactly two consecutive patches' outputs per partition --
     the natural DRAM layout of y.
  3. Four matmuls share one PSUM bank [128, 512]; a single DVE/ACT copy
     moves them to SBUF staging. The host pre-permutes pair columns so
     each output partition holds 16 consecutive pairs, making the store
     a contiguous 8 KB per partition as well.
"""

import numpy as np

import concourse.mybir as mybir
from concourse import bacc
from concourse.bass_utils import run_bass_kernel_spmd
from concourse.tile import TileContext

P = 128   # SBUF partitions
S = 64    # DCT size (contraction dim)
M = 16    # matmuls per macro-tile (128 pairs each)
N_CORES = 8
PAIRS_PER_TILE = P * M          # 2048 pair-columns per macro-tile
PATCHES_PER_TILE = 2 * PAIRS_PER_TILE


def build_kernel(n_patches: int):
    assert n_patches % PATCHES_PER_TILE == 0
    n_tiles = n_patches // PATCHES_PER_TILE
    n_pairs = n_patches // 2
    nc = bacc.Bacc(
        "TRN2",
        target_bir_lowering=False,
        debug=False,
        enable_asserts=False,
        num_devices=N_CORES,
    )
    # xth[(z*64+s), pair] = x[2*pair+z, s], prepared host-side.
    x = nc.dram_tensor("x", [P, n_pairs], mybir.dt.float32, kind="ExternalInput")
    # host-prepared blockdiag(K, K)
    k = nc.dram_tensor("k", [P, P], mybir.dt.float32, kind="ExternalInput")
    y = nc.dram_tensor("y", [n_patches, S], mybir.dt.float32, kind="ExternalOutput")

    xv = x.ap().rearrange("r (t n) -> t r n", n=PAIRS_PER_TILE)
    # device column c = t*2048 + m*128 + q maps to pair p = t*2048 + q*16 + m
    # (host pre-permutes), so out partition q accumulates 16 consecutive
    # pairs -> patch = t*4096 + q*32 + 2m + z and the store is a contiguous
    # 8KB per partition.
    yv = y.ap().rearrange("(t q m z) s -> t q m (z s)", q=P, m=M, z=2)

    with TileContext(nc) as tc:
        with (
            tc.tile_pool(name="consts", bufs=1) as consts,
            tc.tile_pool(name="xin", bufs=6) as x_pool,
            tc.tile_pool(name="outsb", bufs=6) as out_pool,
            tc.tile_pool(name="pout", bufs=6, space="PSUM") as pout_pool,
        ):
            kblk = consts.tile([P, P], mybir.dt.float32)
            first_x = x_pool.tile(
                [P, PAIRS_PER_TILE], mybir.dt.float32, tag="x_tile",
                name="x_head",
            )
            # tile-0 load is emitted first so it heads the Sync queue;
            # kblk rides the otherwise-idle Scalar queue.
            nc.sync.dma_start(out=first_x[:], in_=xv[0])
            nc.scalar.dma_start(out=kblk[:], in_=k.ap())

            for ti in range(n_tiles):
                if ti == 0:
                    x_tile = first_x
                else:
                    x_tile = x_pool.tile(
                        [P, PAIRS_PER_TILE], mybir.dt.float32, tag="x_tile",
                        name=f"x_body{ti}",
                    )
                    nc.sync.dma_start(out=x_tile[:], in_=xv[ti])
                out_sb = out_pool.tile([P, M, P], mybir.dt.float32)
                for g in range(M // 4):
                    po = pout_pool.tile([P, 4, P], mybir.dt.float32)
                    for mm in range(4):
                        m = 4 * g + mm
                        nc.tensor.matmul(
                            po[:, mm, :],
                            lhsT=x_tile[:, P * m : P * (m + 1)],
                            rhs=kblk[:],
                            start=True,
                            stop=True,
                        )
                    if g % 2 == 0:
                        nc.vector.tensor_copy(
                            out_sb[:, 4 * g : 4 * (g + 1), :], po[:]
                        )
                    else:
                        nc.scalar.copy(out_sb[:, 4 * g : 4 * (g + 1), :], po[:])
                # store on the Scalar hwdge queue so it overlaps the
                # Sync-queue input stream
                nc.scalar.dma_start(out=yv[ti], in_=out_sb[:])
    nc.compile()
    return nc


def shard_input(x_core: np.ndarray) -> np.ndarray:
    """[n_patches, 64] -> [128, n_pairs] device layout.

    Device column c = t*2048 + m*128 + q holds pair p = t*2048 + q*16 + m
    (patch = 2p + z), with row r = z*64 + s.
    """
    n = x_core.shape[0]
    t = n // PATCHES_PER_TILE
    x5 = x_core.reshape(t, P, M, 2, S)          # [t, q, m, z, s]
    return np.ascontiguousarray(
        x5.transpose(3, 4, 0, 2, 1).reshape(P, n // 2)  # [(z s), (t m q)]
    )


def kernel(inputs, kernel):
    x_full = np.ascontiguousarray(np.asarray(inputs, dtype=np.float32))
    kmat = np.ascontiguousarray(np.asarray(kernel, dtype=np.float32))
    b, c, h, w = x_full.shape
    assert b == N_CORES, f"expected batch {N_CORES}, got {b}"
    n_patches = c * h * w // S
    nc = build_kernel(n_patches)
    kblk_host = np.zeros((P, P), dtype=np.float32)
    kblk_host[:S, :S] = kmat
    kblk_host[S:, S:] = kmat
    in_maps = [
        {"x": shard_input(x_full[i].reshape(n_patches, S)), "k": kblk_host}
        for i in range(b)
    ]
    res = run_bass_kernel_spmd(nc, in_maps, core_ids=list(range(N_CORES)))
    out = np.stack(
        [res.results[i]["y"].reshape(c, h, w) for i in range(b)], axis=0
    )
    return out


# revision 20
# speedup vs baseline: 1.0319x; 1.0319x over previous
"""Trainium2 Bass kernel for batched 64-point DCT (flattened-patch GEMM).

Reference computation: out = x.reshape(b, -1, 64) @ K, reshaped back.
Pure data parallel over 8 NeuronCores: core i handles batch i as a
[49152, 64] x [64, 64] GEMM.

Sharding strategy: while slicing the batch on the host, each core's input
is also laid out as xth[(z, s), pair] = x[2*pair + z, s] -- a [128, 24576]
matrix whose partition dim is (patch-parity, DCT-coefficient). With that
layout the device kernel needs no on-chip transposes at all:

  1. DMA in [128, 2048] tiles (8 KB contiguous per partition).
  2. One fp32 matmul per 128 pair-columns: stationary = data slice
     [128, 128], moving = blockdiag(K, K) [128, 128]:
       out[q, z*64+f] = sum_s x[2*(base+q)+z, s] * K[s, f]
     which is exactly two consecutive patches' outputs per partition --
     the natural DRAM layout of y.
  3. Four matmuls share one PSUM bank [128, 512]; a single DVE/ACT copy
     moves them to SBUF staging. The host pre-permutes pair columns so
     each output partition holds 16 consecutive pairs, making the store
     a contiguous 8 KB per partition as well.
"""

import numpy as np

import concourse.mybir as mybir
from concourse import bacc
from concourse.bass_utils import run_bass_kernel_spmd
from concourse.tile import TileContext

P = 128   # SBUF partitions
S = 64    # DCT size (contraction dim)
M = 16    # matmuls per macro-tile (128 pairs each)
N_CORES = 8
PAIRS_PER_TILE = P * M          # 2048 pair-columns per macro-tile
PATCHES_PER_TILE = 2 * PAIRS_PER_TILE


def build_kernel(n_patches: int):
    assert n_patches % PATCHES_PER_TILE == 0
    n_tiles = n_patches // PATCHES_PER_TILE
    n_pairs = n_patches // 2
    nc = bacc.Bacc(
        "TRN2",
        target_bir_lowering=False,
        debug=False,
        enable_asserts=False,
        num_devices=N_CORES,
    )
    # xth[(z*64+s), pair] = x[2*pair+z, s], prepared host-side.
    x = nc.dram_tensor("x", [P, n_pairs], mybir.dt.float32, kind="ExternalInput")
    # host-prepared blockdiag(K, K)
    k = nc.dram_tensor("k", [P, P], mybir.dt.float32, kind="ExternalInput")
    y = nc.dram_tensor("y", [n_patches, S], mybir.dt.float32, kind="ExternalOutput")

    xv = x.ap().rearrange("r (t n) -> t r n", n=PAIRS_PER_TILE)
    # device column c = t*2048 + m*128 + q maps to pair p = t*2048 + q*16 + m
    # (host pre-permutes), so out partition q accumulates 16 consecutive
    # pairs -> patch = t*4096 + q*32 + 2m + z and the store is a contiguous
    # 8KB per partition.
    yv = y.ap().rearrange("(t q m z) s -> t q m (z s)", q=P, m=M, z=2)

    with TileContext(nc) as tc:
        with (
            tc.tile_pool(name="consts", bufs=1) as consts,
            tc.tile_pool(name="xin", bufs=6) as x_pool,
            tc.tile_pool(name="outsb", bufs=6) as out_pool,
            tc.tile_pool(name="pout", bufs=8, space="PSUM") as pout_pool,
        ):
            kblk = consts.tile([P, P], mybir.dt.float32)
            first_x = x_pool.tile(
                [P, PAIRS_PER_TILE], mybir.dt.float32, tag="x_tile",
                name="x_head",
            )
            # tile-0 load is emitted first so it heads the Sync queue;
            # kblk rides the otherwise-idle Scalar queue.
            nc.sync.dma_start(out=first_x[:], in_=xv[0])
            nc.scalar.dma_start(out=kblk[:], in_=k.ap())

            for ti in range(n_tiles):
                if ti == 0:
                    x_tile = first_x
                else:
                    x_tile = x_pool.tile(
                        [P, PAIRS_PER_TILE], mybir.dt.float32, tag="x_tile",
                        name=f"x_body{ti}",
                    )
                    nc.sync.dma_start(out=x_tile[:], in_=xv[ti])
                out_sb = out_pool.tile([P, M, P], mybir.dt.float32)
                for g in range(M // 4):
                    po = pout_pool.tile([P, 4, P], mybir.dt.float32)
                    for mm in range(4):
                        m = 4 * g + mm
                        nc.tensor.matmul(
                            po[:, mm, :],
                            lhsT=x_tile[:, P * m : P * (m + 1)],
                            rhs=kblk[:],
                            start=True,
                            stop=True,
                        )
                    if g % 2 == 0:
                        nc.vector.tensor_copy(
                            out_sb[:, 4 * g : 4 * (g + 1), :], po[:]
                        )
                    else:
                        nc.scalar.copy(out_sb[:, 4 * g : 4 * (g + 1), :], po[:])
                # store on the Scalar hwdge queue so it overlaps the
                # Sync-queue input stream
                nc.scalar.dma_start(out=yv[ti], in_=out_sb[:])
    nc.compile()
    return nc


def shard_input(x_core: np.ndarray) -> np.ndarray:
    """[n_patches, 64] -> [128, n_pairs] device layout.

    Device column c = t*2048 + m*128 + q holds pair p = t*2048 + q*16 + m
    (patch = 2p + z), with row r = z*64 + s.
    """
    n = x_core.shape[0]
    t = n // PATCHES_PER_TILE
    x5 = x_core.reshape(t, P, M, 2, S)          # [t, q, m, z, s]
    return np.ascontiguousarray(
        x5.transpose(3, 4, 0, 2, 1).reshape(P, n // 2)  # [(z s), (t m q)]
    )


def kernel(inputs, kernel):
    x_full = np.ascontiguousarray(np.asarray(inputs, dtype=np.float32))
    kmat = np.ascontiguousarray(np.asarray(kernel, dtype=np.float32))
    b, c, h, w = x_full.shape
    assert b == N_CORES, f"expected batch {N_CORES}, got {b}"
    n_patches = c * h * w // S
    nc = build_kernel(n_patches)
    kblk_host = np.zeros((P, P), dtype=np.float32)
    kblk_host[:S, :S] = kmat
    kblk_host[S:, S:] = kmat
    in_maps = [
        {"x": shard_input(x_full[i].reshape(n_patches, S)), "k": kblk_host}
        for i in range(b)
    ]
    res = run_bass_kernel_spmd(nc, in_maps, core_ids=list(range(N_CORES)))
    out = np.stack(
        [res.results[i]["y"].reshape(c, h, w) for i in range(b)], axis=0
    )
    return out
